# revision 14
# baseline (speedup 1.0000x reference)
"""BertMoeLayer Trainium2 kernel: 8-core data-parallel over batch.

Per core (1 batch element, S=1024 tokens, H=768):
  - fp32 attention (QKV, scores, softmax via exp + ones-column denominator,
    ctx, out-proj, LN1) and fp32 router => exact expert routing vs the fp32
    reference (router top-2 gaps go down to ~1e-4; low-precision matmuls flip
    argmax and produce catastrophically wrong expert outputs).
  - On-device token sort by expert (scan-based ranks, indirect-DMA scatter to
    a per-expert-capacity sorted DRAM buffer), If-guarded 128-token blocks
    through a bf16 expert FFN (erf-gelu) + bf16 shared output projection,
    indirect-DMA gather combine, fp32 LN2.

Self-contained: hardcodes all shapes; host side only reshapes/casts inputs.
"""

import numpy as np
import ml_dtypes

import concourse.bass as bass
import concourse.bacc as bacc
import concourse.mybir as mybir
from concourse.bass import make_scalar_value, RegisterHandles
from concourse.bass_utils import run_bass_kernel_spmd
from concourse.masks import make_identity
from concourse.tile import TileContext

F32 = mybir.dt.float32
BF16 = mybir.dt.bfloat16
I32 = mybir.dt.int32
U32 = mybir.dt.uint32
AF = mybir.ActivationFunctionType
OP = mybir.AluOpType

B, S, H = 8, 1024, 768
NH, DH = 12, 64
F, E = 3072, 8
EPS = 1e-12
N_CORES = 8
P = 128
HC = H // P            # 6 chunks of hidden
FC = F // P            # 24 chunks of ffn dim
TC_ = S // P           # 8 token chunks
QTILE = 512
NSLOT = 8              # max 128-token blocks per expert (covers worst case)
CAP = NSLOT * P        # per-expert row capacity in the sorted buffer

_CACHED = {}


def _layernorm(nc, pool, x_sb, g, b, out_sb):
    """LN over last dim (H) of a [P, TC_, H] token-major tile."""
    for t in range(TC_):
        xs = x_sb[:, t, :]
        nmean = pool.tile([P, 1], F32, tag="ln_nm")
        nc.vector.tensor_reduce(nmean[:], xs, axis=mybir.AxisListType.X,
                                op=OP.add, negate=True)
        nc.vector.tensor_scalar_mul(nmean[:], nmean[:], 1.0 / H)
        cent = pool.tile([P, H], F32, tag="ln_cent")
        nc.vector.tensor_scalar_add(cent[:], xs, nmean[:, 0:1])
        sq = pool.tile([P, H], F32, tag="ln_sq")
        ssum = pool.tile([P, 1], F32, tag="ln_ss")
        nc.scalar.square(sq[:], cent[:])
        nc.vector.tensor_reduce(ssum[:], sq[:], axis=mybir.AxisListType.X, op=OP.add)
        nc.vector.tensor_scalar(ssum[:], ssum[:], 1.0 / H, EPS,
                                op0=OP.mult, op1=OP.add)
        nc.scalar.sqrt(ssum[:], ssum[:])
        nc.vector.reciprocal(ssum[:], ssum[:])
        nc.vector.tensor_scalar_mul(cent[:], cent[:], ssum[:, 0:1])
        nc.vector.tensor_tensor(out=cent[:], in0=cent[:], in1=g[:], op=OP.mult)
        nc.vector.tensor_tensor(out=out_sb[:, t, :], in0=cent[:], in1=b[:],
                                op=OP.add)


def _build(debug=False, variant="full"):
    from contextlib import ExitStack
    nc = bacc.Bacc("TRN2", target_bir_lowering=False, debug=False, num_devices=N_CORES)

    # ---- per-core inputs ----
    xT_d = nc.declare_dram_parameter("xT", [P, HC, S], F32, isOutput=False)
    xres_d = nc.declare_dram_parameter("xres", [S, H], F32, isOutput=False)  # x + bao
    # ---- shared weights ----
    wq_d = nc.declare_dram_parameter("wq", [P, HC, H], F32, isOutput=False)
    wk_d = nc.declare_dram_parameter("wk", [P, HC, H], F32, isOutput=False)
    wv_d = nc.declare_dram_parameter("wv", [P, HC, H], F32, isOutput=False)
    wao_d = nc.declare_dram_parameter("wao", [P, HC, H], F32, isOutput=False)
    bqT_d = nc.declare_dram_parameter("bqT", [P, HC], F32, isOutput=False)
    bkT_d = nc.declare_dram_parameter("bkT", [P, HC], F32, isOutput=False)
    bv3_d = nc.declare_dram_parameter("bv3", [P, NH, DH], F32, isOutput=False)
    g1_d = nc.declare_dram_parameter("g1", [P, H], F32, isOutput=False)
    b1_d = nc.declare_dram_parameter("b1", [P, H], F32, isOutput=False)
    wr_d = nc.declare_dram_parameter("wr", [P, HC, E], F32, isOutput=False)
    br_d = nc.declare_dram_parameter("br", [E, 1], F32, isOutput=False)
    wi_d = nc.declare_dram_parameter("wi", [E, P, HC, F], BF16, isOutput=False)
    bi_d = nc.declare_dram_parameter("bi", [E, P, F], BF16, isOutput=False)
    wo_d = nc.declare_dram_parameter("wo", [P, FC, H], BF16, isOutput=False)
    bo_d = nc.declare_dram_parameter("bo", [P, H], F32, isOutput=False)
    g2_d = nc.declare_dram_parameter("g2", [P, H], F32, isOutput=False)
    b2_d = nc.declare_dram_parameter("b2", [P, H], F32, isOutput=False)

    # ---- outputs ----
    out_d = nc.declare_dram_parameter("out", [S, H], F32, isOutput=True)
    rs_d = nc.declare_dram_parameter("rs", [S, E], F32, isOutput=True)
    dbg = {}
    if debug:
        for nm, shp in [("d_a", [S, H]), ("d_logits", [S, E]), ("d_eid", [1, S]),
                        ("d_dest", [1, S]), ("d_counts", [1, E]),
                        ("d_q", [P, HC, S]), ("d_ctx", [P, HC, S]),
                        ("d_inter", [S, H])]:
            dbg[nm] = nc.declare_dram_parameter(nm, shp, F32, isOutput=True)

    # ---- internal DRAM scratch ----
    asort_d = nc.dram_tensor("asort", [E * CAP, H], BF16)
    osort_d = nc.dram_tensor("osort", [E * CAP, H], F32)

    class _SkipRest(Exception):
        pass

    with TileContext(nc) as tc, ExitStack() as root:
      try:
        persist = root.enter_context(tc.tile_pool(name="persist", bufs=1))

        ident = persist.tile([P, P], F32)
        make_identity(nc, ident[:])
        ones1 = persist.tile([1, P], F32)
        nc.vector.memset(ones1[:], 1.0)
        onescol = persist.tile([E, 1], F32)
        nc.vector.memset(onescol[:], 1.0)

        a_sb = persist.tile([P, TC_, H], F32)     # LN1 output, token-major
        dest_tm = persist.tile([P, TC_], I32)     # sorted-slot per token
        counts_row = persist.tile([1, E], I32)

        if variant == "nop":
            with tc.tile_pool(name="nopp", bufs=1) as npool:
                tnop = npool.tile([P, TC_, H], F32)
                nc.sync.dma_start(tnop[:], xres_d[:].rearrange("(t p) h -> p t h", p=P))
                nc.sync.dma_start(out_d[:].rearrange("(t p) h -> p t h", p=P), tnop[:])
                tnop2 = npool.tile([P, TC_, E], F32)
                nc.vector.memset(tnop2[:], 0.0)
                nc.sync.dma_start(rs_d[:].rearrange("(t p) e -> p t e", p=P), tnop2[:])
            raise _SkipRest()

        # ================= Phase 1: attention =================
        stack_ctx = ExitStack()
        ctx_pool = stack_ctx.enter_context(tc.tile_pool(name="ctxp", bufs=1))
        ctxT = ctx_pool.tile([P, HC, S], F32)
        with ExitStack() as phA:   # spans projections..ctx
            pool_qkv = phA.enter_context(tc.tile_pool(name="qkv", bufs=1))
            qT = pool_qkv.tile([P, HC, S], F32)
            kT = pool_qkv.tile([P, HC, S], F32)
            vplus = pool_qkv.tile([P, TC_, NH, DH + 1], F32)

            with ExitStack() as ph1a:
                pool_x = ph1a.enter_context(tc.tile_pool(name="proj_x", bufs=1))
                psA = ph1a.enter_context(tc.tile_pool(name="psA", bufs=4, space="PSUM"))
                xT = pool_x.tile([P, HC, S], F32)
                nc.sync.dma_start(xT[:], xT_d[:])
                pool_in = ph1a.enter_context(tc.tile_pool(name="proj_in", bufs=2))
                wq = pool_in.tile([P, HC, H], F32, tag="w")
                wk = pool_in.tile([P, HC, H], F32, tag="w")
                nc.sync.dma_start(wq[:], wq_d[:])
                nc.sync.dma_start(wk[:], wk_d[:])
                bqT = pool_x.tile([P, HC], F32)
                bkT = pool_x.tile([P, HC], F32)
                bv3 = pool_x.tile([P, NH, DH], F32)
                nc.sync.dma_start(bqT[:], bqT_d[:])
                nc.sync.dma_start(bkT[:], bkT_d[:])
                nc.sync.dma_start(bv3[:], bv3_d[:])

                nc.vector.memset(vplus[:, :, :, DH:DH + 1], 1.0)
                for (w_, b_, o_) in ((wq, bqT, qT), (wk, bkT, kT)):
                    for dc in range(HC):
                        for tq in range(S // QTILE):
                            pq = psA.tile([P, QTILE], F32, tag="ps512")
                            for hc in range(HC):
                                nc.tensor.matmul(
                                    pq[:], lhsT=w_[:, hc, dc * P:(dc + 1) * P],
                                    rhs=xT[:, hc, tq * QTILE:(tq + 1) * QTILE],
                                    start=(hc == 0), stop=(hc == HC - 1))
                            nc.vector.tensor_scalar_add(
                                o_[:, dc, tq * QTILE:(tq + 1) * QTILE],
                                pq[:], b_[:, dc:dc + 1])
                wv = pool_in.tile([P, HC, H], F32, tag="w")
                nc.sync.dma_start(wv[:], wv_d[:])
                for tcv in range(TC_):
                    for ds_, de_ in ((0, 512), (512, 768)):
                        pv = psA.tile([P, QTILE], F32, tag="ps512", name="pv")[:, :de_ - ds_]
                        for hc in range(HC):
                            nc.tensor.matmul(
                                pv[:], lhsT=xT[:, hc, tcv * P:(tcv + 1) * P],
                                rhs=wv[:, hc, ds_:de_],
                                start=(hc == 0), stop=(hc == HC - 1))
                        h0, h1_ = ds_ // DH, de_ // DH
                        nc.vector.tensor_tensor(
                            out=vplus[:, tcv, h0:h1_, 0:DH],
                            in0=pv[:].rearrange("p (h d) -> p h d", d=DH),
                            in1=bv3[:, h0:h1_, :], op=OP.add)
                if debug:
                    nc.sync.dma_start(dbg["d_q"][:], qT[:])
            if variant == "proj":
                raise _SkipRest()

            # ---- scores + exp + ctx ----
            with ExitStack() as ph1b:
                ppool = ph1b.enter_context(tc.tile_pool(name="probs", bufs=2))
                pssc = ph1b.enter_context(tc.tile_pool(name="pssc", bufs=3, space="PSUM"))
                psctx = ph1b.enter_context(tc.tile_pool(name="psctx", bufs=2, space="PSUM"))
                for h in range(NH):
                    hp, hb = h // 2, (h % 2) * DH
                    for tq in range(S // QTILE):
                        qsl = slice(tq * QTILE, (tq + 1) * QTILE)
                        probs = ppool.tile([P, TC_, QTILE], F32, tag="probs")
                        for kc in range(TC_):
                            psc = pssc.tile([P, QTILE], F32, tag="psc")
                            nc.tensor.matmul(
                                psc[:],
                                lhsT=kT[hb:hb + DH, hp, kc * P:(kc + 1) * P],
                                rhs=qT[hb:hb + DH, hp, qsl],
                                start=True, stop=True, tile_position=(hb, 0))
                            nc.scalar.activation(probs[:, kc, :], psc[:], AF.Exp)
                        pc = psctx.tile([DH + 1, QTILE], F32, tag="pc")
                        for kc in range(TC_):
                            nc.tensor.matmul(
                                pc[:], lhsT=vplus[:, kc, h, :],
                                rhs=probs[:, kc, :],
                                start=(kc == 0), stop=(kc == TC_ - 1))
                        denr = ppool.tile([1, QTILE], F32, tag="denr")
                        nc.vector.reciprocal(denr[:], pc[DH:DH + 1, :])
                        pd = psctx.tile([DH, QTILE], F32, tag="pd")
                        nc.tensor.matmul(pd[:], lhsT=ones1[:, :DH], rhs=denr[:],
                                         start=True, stop=True)
                        dsb = ppool.tile([DH, QTILE], F32, tag="dsb")
                        nc.vector.tensor_copy(dsb[:], pd[:])
                        nc.vector.tensor_tensor(
                            out=ctxT[hb:hb + DH, hp, qsl],
                            in0=pc[0:DH, :], in1=dsb[:], op=OP.mult)
            if debug:
                nc.sync.dma_start(dbg["d_ctx"][:], ctxT[:])
            if variant == "ctx":
                raise _SkipRest()

        # ---- Wao + residual + LN1 ----
        with ExitStack() as phC:
            pool_ao = phC.enter_context(tc.tile_pool(name="ao", bufs=1))
            psC = phC.enter_context(tc.tile_pool(name="psC", bufs=4, space="PSUM"))
            wao = pool_ao.tile([P, HC, H], F32)
            nc.sync.dma_start(wao[:], wao_d[:])
            xres = pool_ao.tile([P, TC_, H], F32)
            nc.sync.dma_start(xres[:], xres_d[:].rearrange("(t p) h -> p t h", p=P))
            g1 = pool_ao.tile([P, H], F32)
            b1 = pool_ao.tile([P, H], F32)
            nc.sync.dma_start(g1[:], g1_d[:])
            nc.sync.dma_start(b1[:], b1_d[:])

            pre = pool_ao.tile([P, TC_, H], F32)
            for tcv in range(TC_):
                for ds_, de_ in ((0, 512), (512, 768)):
                    pa = psC.tile([P, QTILE], F32, tag="ps512", name="pa")[:, :de_ - ds_]
                    for dc in range(HC):
                        nc.tensor.matmul(
                            pa[:], lhsT=ctxT[:, dc, tcv * P:(tcv + 1) * P],
                            rhs=wao[:, dc, ds_:de_],
                            start=(dc == 0), stop=(dc == HC - 1))
                    nc.vector.tensor_tensor(
                        out=pre[:, tcv, ds_:de_], in0=pa[:],
                        in1=xres[:, tcv, ds_:de_], op=OP.add)
            if variant == "wao":
                raise _SkipRest()
            _layernorm(nc, pool_ao, pre, g1, b1, a_sb)
            if debug:
                nc.sync.dma_start(
                    dbg["d_a"][:].rearrange("(t p) h -> p t h", p=P), a_sb[:])

        stack_ctx.close()  # free ctxT before router/sort phase

        if variant == "ln1":
            raise _SkipRest()

        # ---- router + sort tables + dispatch ----
        with ExitStack() as phD:
            pool = phD.enter_context(tc.tile_pool(name="rt2", bufs=1))
            abf_sb = pool.tile([P, TC_, H], BF16)
            nc.vector.tensor_copy(abf_sb[:], a_sb[:])
            psD = phD.enter_context(tc.tile_pool(name="psD", bufs=2, space="PSUM"))
            psD1 = phD.enter_context(tc.tile_pool(name="psD1", bufs=1, space="PSUM"))

            aT = pool.tile([P, HC, S], F32)
            for tcv in range(TC_):
                for hc in range(HC):
                    pt = psD.tile([P, P], F32, tag="pt")
                    nc.tensor.transpose(
                        pt[:], a_sb[:, tcv, hc * P:(hc + 1) * P], ident[:])
                    nc.vector.tensor_copy(aT[:, hc, tcv * P:(tcv + 1) * P], pt[:])

            wr = pool.tile([P, HC, E], F32)
            br = pool.tile([E, 1], F32)
            nc.sync.dma_start(wr[:], wr_d[:])
            nc.sync.dma_start(br[:], br_d[:])
            logitsT = pool.tile([E, S], F32)
            for tq in range(S // QTILE):
                pr = psD.tile([E, QTILE], F32, tag="pt")
                for hc in range(HC):
                    nc.tensor.matmul(
                        pr[:], lhsT=wr[:, hc, :],
                        rhs=aT[:, hc, tq * QTILE:(tq + 1) * QTILE],
                        start=(hc == 0), stop=(hc == HC - 1))
                nc.vector.tensor_scalar_add(
                    logitsT[:, tq * QTILE:(tq + 1) * QTILE], pr[:], br[:, 0:1])

            logits_tm = pool.tile([P, TC_, E], F32)
            for tcv in range(TC_):
                pt = psD.tile([P, P], F32, tag="pt")
                nc.tensor.transpose(
                    pt[:, :E], logitsT[:, tcv * P:(tcv + 1) * P], ident[:E, :E])
                nc.vector.tensor_copy(logits_tm[:, tcv, :], pt[:, :E])
            if debug:
                nc.sync.dma_start(
                    dbg["d_logits"][:].rearrange("(t p) e -> p t e", p=P),
                    logits_tm[:])

            rs_sb = pool.tile([P, TC_, E], F32)
            eid_f = pool.tile([P, TC_], F32)
            for tcv in range(TC_):
                mx = pool.tile([P, 8], F32, tag="mx")
                mi = pool.tile([P, 8], U32, tag="mi")
                nc.vector.max(mx[:], logits_tm[:, tcv, :])
                nc.vector.max_index(mi[:], mx[:], logits_tm[:, tcv, :])
                nc.vector.tensor_copy(eid_f[:, tcv:tcv + 1], mi[:, 0:1])
                sm = pool.tile([P, E], F32, tag="sm")
                nc.vector.tensor_scalar(
                    sm[:], logits_tm[:, tcv, :], mx[:, 0:1], None, op0=OP.subtract)
                nc.scalar.activation(sm[:], sm[:], AF.Exp)
                ssum = pool.tile([P, 1], F32, tag="ssum")
                nc.vector.tensor_reduce(ssum[:], sm[:], axis=mybir.AxisListType.X,
                                        op=OP.add)
                nc.vector.reciprocal(ssum[:], ssum[:])
                nc.vector.tensor_scalar(
                    rs_sb[:, tcv, :], sm[:], ssum[:, 0:1], None, op0=OP.mult)
            nc.sync.dma_start(rs_d[:].rearrange("(t p) e -> p t e", p=P), rs_sb[:])

            # eid as [1, S] row
            eid_row = pool.tile([1, S], F32)
            pt8 = psD.tile([TC_, P], F32, tag="pt")
            nc.tensor.transpose(pt8[:], eid_f[:], ident[:])
            eid_c = pool.tile([TC_, P], F32)
            nc.vector.tensor_copy(eid_c[:], pt8[:])
            nc.sync.dma_start(
                eid_row[:].rearrange("o (c p) -> o c p", c=TC_), eid_c[:, None, :])
            if debug:
                nc.sync.dma_start(dbg["d_eid"][:], eid_row[:])

            eid_rep = pool.tile([E, S], F32)
            for half in range(2):
                sl = slice(half * 512, half * 512 + 512)
                pe_ = psD.tile([E, 512], F32, tag="pt")
                nc.tensor.matmul(pe_[:], lhsT=ones1[:, :E], rhs=eid_row[:, sl],
                                 start=True, stop=True)
                nc.vector.tensor_copy(eid_rep[:, sl], pe_[:])

            iot = pool.tile([E, 1], I32)
            nc.gpsimd.iota(iot[:], pattern=[[0, 1]], base=0, channel_multiplier=1)
            iotf = pool.tile([E, 1], F32)
            nc.vector.tensor_copy(iotf[:], iot[:])
            mask = pool.tile([E, S], F32)
            nc.vector.tensor_tensor(
                out=mask[:], in0=eid_rep[:],
                in1=iotf[:, 0:1].to_broadcast([E, S]), op=OP.is_equal)
            onesE = pool.tile([E, S], F32)
            nc.vector.memset(onesE[:], 1.0)
            cum = pool.tile([E, S], F32)
            nc.vector.tensor_tensor_scan(
                cum[:], mask[:], onesE[:], 0.0, op0=OP.add, op1=OP.mult)
            rankm = pool.tile([E, S], F32)
            nc.vector.tensor_tensor(out=rankm[:], in0=cum[:], in1=mask[:], op=OP.mult)
            nc.vector.tensor_tensor(out=rankm[:], in0=rankm[:], in1=mask[:],
                                    op=OP.subtract)
            cnt_colI = pool.tile([E, 1], I32)
            nc.vector.tensor_copy(cnt_colI[:], cum[:, S - 1:S])
            nc.sync.dma_start(counts_row[:], cnt_colI[:, 0:1])
            if debug:
                cntf = pool.tile([1, E], F32)
                nc.vector.tensor_copy(cntf[:], counts_row[:])
                nc.sync.dma_start(dbg["d_counts"][:], cntf[:])

            destps = psD1.tile([1, S], F32, tag="destps")
            for half in range(2):
                sl = slice(half * 512, half * 512 + 512)
                nc.tensor.matmul(destps[:, sl], lhsT=onescol[:],
                                 rhs=rankm[:, sl], start=True, stop=True)
            dest_row = pool.tile([1, S], F32)
            nc.vector.scalar_tensor_tensor(
                out=dest_row[:], in0=eid_row[:], scalar=float(CAP),
                in1=destps[:], op0=OP.mult, op1=OP.add)
            if debug:
                nc.sync.dma_start(dbg["d_dest"][:], dest_row[:])

            dest_tmf = pool.tile([P, TC_], F32)
            for tcv in range(TC_):
                ptd = psD.tile([P, P], F32, tag="pt")
                nc.tensor.transpose(
                    ptd[:, 0:1], dest_row[:, tcv * P:(tcv + 1) * P], ident[:1, :1])
                nc.vector.tensor_copy(dest_tmf[:, tcv:tcv + 1], ptd[:, 0:1])
            nc.vector.tensor_copy(dest_tm[:], dest_tmf[:])

            if variant == "sort":
                raise _SkipRest()
            # dispatch: scatter bf16 LN1 rows into sorted order
            for tcv in range(TC_):
                nc.gpsimd.indirect_dma_start(
                    out=asort_d[:],
                    out_offset=bass.IndirectOffsetOnAxis(
                        ap=dest_tm[:, tcv:tcv + 1], axis=0),
                    in_=abf_sb[:, tcv, :], in_offset=None)

        # ================= Phase 2: expert FFN =================
        with ExitStack() as ph3:
            if variant == "nof":
                raise _SkipRest()
            wop = ph3.enter_context(tc.tile_pool(name="wop", bufs=1))
            wpool = ph3.enter_context(tc.tile_pool(name="wpool", bufs=2))
            bpool = ph3.enter_context(tc.tile_pool(name="blk", bufs=2))
            psff = ph3.enter_context(tc.tile_pool(name="psff", bufs=3, space="PSUM"))
            psfo = ph3.enter_context(tc.tile_pool(name="psfo", bufs=2, space="PSUM"))

            wo = wop.tile([P, FC, H], BF16)
            nc.sync.dma_start(wo[:], wo_d[:])

            cvals = []
            for e in range(E):
                regs = []
                for eng in [mybir.EngineType.Activation, mybir.EngineType.DVE,
                            mybir.EngineType.PE, mybir.EngineType.Pool,
                            mybir.EngineType.SP]:
                    r = nc.alloc_register(eng, f"cnt{e}_{eng.name}")
                    nc.reg_load(r, counts_row[0:1, e:e + 1])
                    regs.append(r)
                cvals.append(make_scalar_value(RegisterHandles(regs),
                                               min_val=0, max_val=S))

            def ffn_slot(e, j, wbuf, bib):
                rbase = e * CAP + j * P
                ab = bpool.tile([P, H], BF16, tag="ab")
                nc.sync.dma_start(ab[:], asort_d[rbase:rbase + P, :])
                abT = bpool.tile([P, HC, P], BF16, tag="abT")
                nc.sync.dma_start_transpose(abT[:], ab[:])
                h1 = bpool.tile([P, F], BF16, tag="h1")
                for fc in range(F // 512):
                    pf = psff.tile([P, 512], F32, tag="pf")
                    for hc in range(HC):
                        nc.tensor.matmul(
                            pf[:], lhsT=abT[:, hc, :],
                            rhs=wbuf[:, hc, fc * 512:(fc + 1) * 512],
                            start=(hc == 0), stop=(hc == HC - 1))
                    nc.vector.tensor_tensor(
                        out=pf[:], in0=pf[:],
                        in1=bib[:, fc * 512:(fc + 1) * 512], op=OP.add)
                    nc.scalar.activation(
                        h1[:, fc * 512:(fc + 1) * 512], pf[:], AF.Gelu)
                h1T = bpool.tile([P, FC, P], BF16, tag="h1T")
                nc.sync.dma_start_transpose(h1T[:], h1[:])
                osb = bpool.tile([P, H], F32, tag="osb")
                for ds_, de_ in ((0, 512), (512, 768)):
                    po = psfo.tile([P, 512], F32, tag="po", name="po")[:, :de_ - ds_]
                    for fc in range(FC):
                        nc.tensor.matmul(
                            po[:], lhsT=h1T[:, fc, :], rhs=wo[:, fc, ds_:de_],
                            start=(fc == 0), stop=(fc == FC - 1))
                    nc.vector.tensor_copy(osb[:, ds_:de_], po[:])
                nc.sync.dma_start(osort_d[rbase:rbase + P, :], osb[:])

            for e in range(E):
                wbuf = wpool.tile([P, HC, F], BF16, tag="wbuf")
                nc.sync.dma_start(wbuf[:], wi_d[e])
                bib = wpool.tile([P, F], BF16, tag="bib")
                nc.sync.dma_start(bib[:], bi_d[e])

                if variant == "noguard":
                    ffn_slot(e, 0, wbuf, bib)
                    ffn_slot(e, 1, wbuf, bib)
                else:
                    def nest(j, e=e, wbuf=wbuf, bib=bib):
                        with tc.If(cvals[e] > j * P):
                            ffn_slot(e, j, wbuf, bib)
                            if j + 1 < NSLOT:
                                nest(j + 1)
                    nest(0)

        # ================= Phase 3: combine + LN2 =================
        with ExitStack() as ph4:
            if variant == "noc":
                raise _SkipRest()
            pool4 = ph4.enter_context(tc.tile_pool(name="fin", bufs=2))
            cpool = ph4.enter_context(tc.tile_pool(name="cp", bufs=1))
            bo = cpool.tile([P, H], F32)
            g2 = cpool.tile([P, H], F32)
            b2 = cpool.tile([P, H], F32)
            nc.sync.dma_start(bo[:], bo_d[:])
            nc.sync.dma_start(g2[:], g2_d[:])
            nc.sync.dma_start(b2[:], b2_d[:])

            pre2 = cpool.tile([P, TC_, H], F32)
            for tcv in range(TC_):
                inter = pool4.tile([P, H], F32, tag="inter")
                nc.gpsimd.indirect_dma_start(
                    out=inter[:], out_offset=None, in_=osort_d[:],
                    in_offset=bass.IndirectOffsetOnAxis(
                        ap=dest_tm[:, tcv:tcv + 1], axis=0))
                if debug:
                    nc.sync.dma_start(
                        dbg["d_inter"][tcv * P:(tcv + 1) * P, :], inter[:])
                nc.vector.tensor_tensor(out=pre2[:, tcv, :], in0=inter[:],
                                        in1=a_sb[:, tcv, :], op=OP.add)
                nc.vector.tensor_tensor(out=pre2[:, tcv, :], in0=pre2[:, tcv, :],
                                        in1=bo[:], op=OP.add)
            outt = cpool.tile([P, TC_, H], F32)
            _layernorm(nc, cpool, pre2, g2, b2, outt)
            nc.sync.dma_start(out_d[:].rearrange("(t p) h -> p t h", p=P), outt[:])

      except _SkipRest:
        try:
            stack_ctx.close()
        except Exception:
            pass
    nc.compile()
    return nc


def _prep_host(inputs):
    f32 = lambda x: np.ascontiguousarray(np.asarray(x, dtype=np.float32))
    hs = f32(inputs["hidden_states"])
    Wq = f32(inputs["Wq"]) / np.float32(8.0)
    bq = f32(inputs["bq"]) / np.float32(8.0)
    Wk, bk = f32(inputs["Wk"]), f32(inputs["bk"])
    Wv, bv = f32(inputs["Wv"]), f32(inputs["bv"])
    Wao, bao = f32(inputs["Wao"]), f32(inputs["bao"])
    Wr, br = f32(inputs["Wr"]), f32(inputs["br"])
    Wi, bi = f32(inputs["Wi"]), f32(inputs["bi"])
    Wo, bo = f32(inputs["Wo"]), f32(inputs["bo"])

    def chunk6(w):  # [768, X] -> [128, 6, X]
        return np.ascontiguousarray(w.reshape(HC, P, -1).transpose(1, 0, 2))

    shared = {
        "wq": chunk6(Wq), "wk": chunk6(Wk), "wv": chunk6(Wv), "wao": chunk6(Wao),
        "bqT": np.ascontiguousarray(bq.reshape(HC, P).T),
        "bkT": np.ascontiguousarray(bk.reshape(HC, P).T),
        "bv3": np.ascontiguousarray(np.broadcast_to(bv.reshape(NH, DH), (P, NH, DH))),
        "g1": np.ascontiguousarray(np.broadcast_to(f32(inputs["ln1_g"]), (P, H))),
        "b1": np.ascontiguousarray(np.broadcast_to(f32(inputs["ln1_b"]), (P, H))),
        "wr": chunk6(Wr), "br": np.ascontiguousarray(br.reshape(E, 1)),
        "wi": np.ascontiguousarray(
            Wi.reshape(E, HC, P, F).transpose(0, 2, 1, 3).astype(ml_dtypes.bfloat16)),
        "bi": np.ascontiguousarray(
            np.broadcast_to(bi[:, None, :], (E, P, F)).astype(ml_dtypes.bfloat16)),
        "wo": np.ascontiguousarray(
            Wo.reshape(FC, P, H).transpose(1, 0, 2).astype(ml_dtypes.bfloat16)),
        "bo": np.ascontiguousarray(np.broadcast_to(bo, (P, H))),
        "g2": np.ascontiguousarray(np.broadcast_to(f32(inputs["ln2_g"]), (P, H))),
        "b2": np.ascontiguousarray(np.broadcast_to(f32(inputs["ln2_b"]), (P, H))),
    }
    in_maps = []
    for c in range(N_CORES):
        x = hs[c]
        m = dict(shared)
        m["xT"] = np.ascontiguousarray(x.T.reshape(HC, P, S).transpose(1, 0, 2))
        m["xres"] = np.ascontiguousarray(x + bao)
        in_maps.append(m)
    return in_maps


def run(inputs, debug=False, trace=False, variant="full"):
    key = ("dbg" if debug else "plain", variant)
    if key not in _CACHED:
        _CACHED[key] = _build(debug=debug, variant=variant)
    nc = _CACHED[key]
    in_maps = _prep_host(inputs)
    return run_bass_kernel_spmd(nc, in_maps, list(range(N_CORES)), trace=trace)


def kernel(**inputs):
    res = run(inputs)
    lo = np.stack([np.asarray(res.results[c]["out"]) for c in range(N_CORES)])
    rs = np.stack([np.asarray(res.results[c]["rs"]) for c in range(N_CORES)])
    return lo.astype(np.float32), rs.astype(np.float32)
